# revision 1
# baseline (speedup 1.0000x reference)
"""HardTripletLoss on 8 Trainium2 NeuronCores (Bass/Tile).

Math
----
reference: emb = l2_normalize(embeddings); dist = cdist(emb, emb);
  pos_stat[i] = mean_{j: same class, j!=i} dist[i,j]
  neg_stat[i] = min_{j: diff class} dist[i,j]
  loss = mean over valid rows of relu(pos_stat - neg_stat + 1)

For unit vectors dist^2 = 2 - 2*ghat where ghat = N @ N.T.  We fold the
class mask into the GEMM itself: with Y = onehot(labels) [64, B],

  A = [ N.T ; -Y ]  (rhs side; the lhsT +2*Y block ships separately)

and contracting lhsT = [N.T ; +2*Y] (the +2*Y block from a separate small
per-core tile) against rhs = [N.T ; -Y] gives
P = ghat - 2*S  (S = same-class indicator incl diagonal).  Then per row:
  masked positive dists = sqrt(2*relu(-1 - P))   (diff-class & diagonal -> 0)
  hardest negative      = sqrt(relu(2 - 2*rowmax(P)))
      (rowmax(P) = max over diff-class ghat, since same-class P <= -1+eps)

Sharding: rows split 512/core (data parallel).  Every core holds all 4096
columns of A in SBUF (10.5 MB) as 8 slabs of 512 columns; slab order is
rotated per core so each core's first-loaded slab contains its own shard
columns (the matmul stationary operand), letting the GEMM start after the
first 1.3 MB DMA.  Row stats are order-invariant (sum/max over columns).

GEMM operands are bf16 (fast weight load, half DMA; fp32 PSUM accumulate;
measured end-to-end rel err ~1.3e-4).  The epilogue per PSUM chunk is one
clamp (ACT relu or DVE min, alternating for balance), one ACT sqrt whose
accum_out emits the row-sum for free, and one DVE row-max.  Dummy warm-up
matmuls open the PE HAM clock gate during the first slab DMA.

Host does only input marshaling (normalize+transpose+onehot packing,
O(B*D), 0.02% of the FLOPs) and the final O(B) per-row combine + mean
over the device-computed row statistics.
"""

import sys

if "/opt/trn_rl_repo" not in sys.path:
    sys.path.insert(0, "/opt/trn_rl_repo")

import ml_dtypes
import numpy as np


import concourse.bass as bass
import concourse.bacc as bacc
import concourse.mybir as mybir
import concourse.tile as tile
from concourse.bass_utils import run_bass_kernel_spmd

F32 = mybir.dt.float32
F32R = mybir.dt.float32r
BF16 = mybir.dt.bfloat16
GEMM_DT = BF16  # bf16: fast weight load + half DMA; f32r fallback if accuracy demands
ALU = mybir.AluOpType
ACTF = mybir.ActivationFunctionType
AXX = mybir.AxisListType.X

B = 4096
D = 512
C = 64
NCORES = 8
SHARD = B // NCORES          # 512 rows per core
MT = SHARD // 128            # 4 m-tiles per core
NJ = 8                       # column slabs of 512
KC = 5                       # k-chunks of 128 (4 data + 1 +/- onehot)
SLABW = KC * 512             # 2560
# psum chunk structure: slabs 0 and 1 get single-slab chunks (they arrive
# first and pace the pipeline start); later slabs are paired for bigger,
# cheaper epilogue ops.  One chunk = (slab set, m-tile) -> one PSUM tile.
JSETS = [[0], [1], [2, 3], [4, 5], [6, 7]]
MT_ = 4
CHUNKS = [(js, m) for js in JSETS for m in range(MT_)]
NCHUNK = len(CHUNKS)         # 20

MARGIN = 1.0


def _build_nc():
    nc = bacc.Bacc(
        "TRN2",
        target_bir_lowering=False,
        debug=False,
        enable_asserts=False,
        num_devices=NCORES,
    )
    atp = nc.dram_tensor("atp", [NJ, 128, SLABW], GEMM_DT, kind="ExternalInput")
    yl = nc.dram_tensor("yl", [C, SHARD], GEMM_DT, kind="ExternalInput")
    NCOL = NCHUNK + 1  # +1: last chunk's epilogue runs as two halves
    stats_d = nc.dram_tensor("stats", [128, 2 * NCOL], F32, kind="ExternalOutput")

    with tile.TileContext(nc) as tc:
        with (
            tc.tile_pool(name="slabs", bufs=1) as slabs,
            tc.tile_pool(name="psum", bufs=4, space=bass.MemorySpace.PSUM) as psum,
            tc.tile_pool(name="scr", bufs=3) as scr,
            tc.tile_pool(name="stat", bufs=1) as stat,
        ):
            # small lhsT-side one-hot block first: every group's c=4 matmul
            # needs it, so it must not queue behind 10 MB of slab DMA
            ylt = stat.tile([C, SHARD], GEMM_DT, name="ylt", tag="ylt")
            nc.sync.dma_start(ylt[:], yl.ap())
            # slab 0 (the stationary-operand columns) split into per-k-chunk
            # pieces so the first matmuls start after ~130 KB, not 650 KB
            s0c = []
            for c in range(KC):
                t = slabs.tile([128, 512], GEMM_DT, name=f"s0c{c}", tag=f"s0c{c}")
                nc.sync.dma_start(t[:], atp[0, :, c * 512 : (c + 1) * 512])
                s0c.append(t)
            slab_t = [None]
            for j in range(1, NJ):
                t = slabs.tile([128, SLABW], GEMM_DT, name=f"slab{j}", tag=f"slab{j}")
                nc.sync.dma_start(t[:], atp[j])
                slab_t.append(t)

            # per-(m, group) partial stats in one tile: cols [0, NCOL) pos
            # row-sums, cols [NCOL, 2*NCOL) row-maxes
            parts = stat.tile([128, 2 * NCOL], F32, name="parts", tag="parts")

            # bias constants for ACT (float biases need pre-registered const
            # APs, so build [128,1] tiles explicitly)
            bias_c = {}
            for bname, bval in [("m1", -1.0), ("m2", -2.0), ("z", 0.0)]:
                bt = stat.tile([128, 1], F32, name=f"bc_{bname}", tag=f"bc_{bname}")
                nc.gpsimd.memset(bt[:], bval)
                bias_c[bname] = bt

            warm = stat.tile([128, 1], F32, name="warm", tag="warm")
            nc.scalar.activation(warm[:], bias_c["z"][:], ACTF.Relu)
            nc.scalar.activation(warm[:], warm[:], ACTF.Sqrt, bias=bias_c["z"][:])

            # PE warm-up: ~4us of dummy matmuls while the first slab DMA is in
            # flight, so the HAM clock-gate opens before the real GEMM starts
            warm_w = stat.tile([128, 128], GEMM_DT, name="warm_w", tag="warm_w")
            warm_x = stat.tile([128, 512], GEMM_DT, name="warm_x", tag="warm_x")
            nc.gpsimd.memset(warm_w[:], 0.0)
            nc.gpsimd.memset(warm_x[:], 0.0)
            wpt = psum.tile([128, 512], F32, name="wpt", tag="pt")
            for _ in range(9):
                nc.tensor.matmul(wpt[:], warm_w[:], warm_x[:], start=True, stop=True)

            for k, (jset, m) in enumerate(CHUNKS):
                w = len(jset) * 512
                pt = psum.tile([128, w], F32, name="pt", tag="pt")
                for ci, c in enumerate(range(KC)):
                    for jj, j in enumerate(jset):
                        if c < KC - 1:
                            lhsT = s0c[c][:, m * 128 : (m + 1) * 128]
                            rhs = (
                                s0c[c][:, :]
                                if j == 0
                                else slab_t[j][:, c * 512 : (c + 1) * 512]
                            )
                        else:
                            lhsT = ylt[:, m * 128 : (m + 1) * 128]
                            rhs = (
                                s0c[c][0:C, :]
                                if j == 0
                                else slab_t[j][0:C, c * 512 : (c + 1) * 512]
                            )
                        nc.tensor.matmul(
                            pt[:, jj * 512 : (jj + 1) * 512],
                            lhsT,
                            rhs,
                            start=(ci == 0),
                            stop=(ci == KC - 1),
                        )
                last = k == NCHUNK - 1
                t1 = scr.tile([128, 1024], F32, name="t1", tag="t1")
                d1 = scr.tile([128, 1024], F32, name="d1", tag="d1")
                if last:
                    # two halves, clamp on ACT and DVE in parallel, to
                    # shorten the end-of-kernel serial chain
                    h = w // 2
                    nc.scalar.activation(
                        t1[:, :h], pt[:, :h], ACTF.Relu,
                        bias=bias_c["m1"][:], scale=-1.0,
                    )
                    nc.vector.tensor_scalar(
                        t1[:, h:w], pt[:, h:], -1.0, None, op0=ALU.min
                    )
                    nc.scalar.activation(
                        d1[:, :h], t1[:, :h], ACTF.Sqrt,
                        bias=bias_c["z"][:], scale=2.0,
                        accum_out=parts[:, k : k + 1],
                    )
                    nc.scalar.activation(
                        d1[:, h:w], t1[:, h:w], ACTF.Sqrt,
                        bias=bias_c["m2"][:], scale=-2.0,
                        accum_out=parts[:, k + 1 : k + 2],
                    )
                    nc.vector.tensor_reduce(
                        parts[:, NCOL + k : NCOL + k + 1],
                        pt[:, :h], axis=AXX, op=ALU.max,
                    )
                    nc.vector.tensor_reduce(
                        parts[:, NCOL + k + 1 : NCOL + k + 2],
                        pt[:, h:], axis=AXX, op=ALU.max,
                    )
                else:
                    if k % 2 == 0:
                        # ACT: t1 = relu(-P - 1); d = sqrt(2*t1)
                        nc.scalar.activation(
                            t1[:, :w], pt[:], ACTF.Relu,
                            bias=bias_c["m1"][:], scale=-1.0,
                        )
                        nc.scalar.activation(
                            d1[:, :w], t1[:, :w], ACTF.Sqrt,
                            bias=bias_c["z"][:], scale=2.0,
                            accum_out=parts[:, k : k + 1],
                        )
                    else:
                        # DVE: t1 = min(P, -1); d = sqrt(-2*t1 - 2)
                        nc.vector.tensor_scalar(
                            t1[:, :w], pt[:], -1.0, None, op0=ALU.min
                        )
                        nc.scalar.activation(
                            d1[:, :w], t1[:, :w], ACTF.Sqrt,
                            bias=bias_c["m2"][:], scale=-2.0,
                            accum_out=parts[:, k : k + 1],
                        )
                    nc.vector.tensor_reduce(
                        parts[:, NCOL + k : NCOL + k + 1],
                        pt[:], axis=AXX, op=ALU.max,
                    )

            nc.sync.dma_start(stats_d.ap(), parts[:])

    nc.compile()
    return nc


_NC_CACHE: dict = {}


def _get_nc():
    if "nc" not in _NC_CACHE:
        _NC_CACHE["nc"] = _build_nc()
    return _NC_CACHE["nc"]


def _prep_inputs(embeddings: np.ndarray, labels: np.ndarray):
    E = np.asarray(embeddings, dtype=np.float32)
    L = np.asarray(labels).astype(np.int64)
    assert E.shape == (B, D) and L.shape == (B,)

    nrm = np.maximum(np.linalg.norm(E.astype(np.float32), axis=1), 1e-12)
    N = (E / nrm[:, None].astype(np.float32)).astype(np.float32)

    Y = (L[None, :] == np.arange(C, dtype=np.int64)[:, None]).astype(np.float32)
    # chunk 4 partitions 0:64 hold -Y (the rhs side); the +2*Y lhsT side
    # ships separately per core (yl).  Partitions 64:128 stay zero.
    AT = np.zeros((KC * 128, B), dtype=np.float32)
    AT[:D] = N.T
    AT[D : D + C] = -Y

    # slabs[j][p, c*512+x] = AT[128c+p, 512j+x]
    slabs8 = np.ascontiguousarray(
        AT.reshape(KC, 128, NJ, 512)
        .transpose(2, 1, 0, 3)
        .reshape(NJ, 128, SLABW)
        .astype(ml_dtypes.bfloat16)
    )

    cnt = np.bincount(L, minlength=C)
    pos_cnt = cnt[L] - 1
    neg_cnt = B - cnt[L]
    invc = (1.0 / np.maximum(pos_cnt, 1)).astype(np.float32)
    valid = ((pos_cnt > 0) & (neg_cnt > 0)).astype(np.float32)

    in_maps = []
    for r in range(NCORES):
        rows = slice(SHARD * r, SHARD * (r + 1))
        in_maps.append(
            {
                "atp": np.ascontiguousarray(np.roll(slabs8, -r, axis=0)),
                "yl": np.ascontiguousarray((2.0 * Y[:, rows]).astype(ml_dtypes.bfloat16)),
            }
        )
    return in_maps, (invc, valid)


def _finish(results, aux):
    invc, valid = aux
    NCOL = NCHUNK + 1
    pos_sum = np.empty(B, dtype=np.float32)
    max_p = np.empty(B, dtype=np.float32)
    for r in range(NCORES):
        st = np.asarray(results[r]["stats"])
        pp, mp = st[:, :NCOL], st[:, NCOL:]
        psum_m = np.zeros((128, MT), dtype=np.float32)
        pmax_m = np.full((128, MT), -np.inf, dtype=np.float32)
        for k, (jset, m) in enumerate(CHUNKS):
            psum_m[:, m] += pp[:, k]
            pmax_m[:, m] = np.maximum(pmax_m[:, m], mp[:, k])
        # split last chunk's second half lives in the extra column
        m_last = CHUNKS[-1][1]
        psum_m[:, m_last] += pp[:, NCHUNK]
        pmax_m[:, m_last] = np.maximum(pmax_m[:, m_last], mp[:, NCHUNK])
        rows = slice(SHARD * r, SHARD * (r + 1))
        pos_sum[rows] = psum_m.T.reshape(SHARD)
        max_p[rows] = pmax_m.T.reshape(SHARD)
    pos_stat = pos_sum * invc
    neg_stat = np.sqrt(np.maximum(2.0 - 2.0 * max_p, 0.0), dtype=np.float32)
    per_row = np.maximum(pos_stat - neg_stat + MARGIN, 0.0) * valid
    n_valid = float(valid.sum())
    total = float(per_row.sum(dtype=np.float32))
    out = total / max(n_valid, 1.0) if n_valid > 0 else 0.0
    return np.array(out, dtype=np.float32)


def kernel(embeddings, labels, _run_kwargs=None):
    nc = _get_nc()
    in_maps, aux = _prep_inputs(embeddings, labels)
    res = run_bass_kernel_spmd(
        nc, in_maps, core_ids=list(range(NCORES)), **(_run_kwargs or {})
    )
    out = _finish(res.results, aux)
    if _run_kwargs:
        return out, res
    return out



# revision 8
# speedup vs baseline: 1.4456x; 1.4456x over previous
"""HardTripletLoss on 8 Trainium2 NeuronCores (Bass/Tile), fp8 edition.

Math
----
reference: emb = l2_normalize(embeddings); dist = cdist(emb, emb);
  pos_stat[i] = mean_{j: same class, j!=i} dist[i,j]
  neg_stat[i] = min_{j: diff class} dist[i,j]
  loss = mean over valid rows of relu(pos_stat - neg_stat + 1)

For unit vectors dist^2 = 2 - 2*ghat with ghat = N @ N.T.  We quantize
X = 8*N to fp8e4m3 (measured end-to-end rel err ~5e-4) and fold the
class mask into the GEMM: P = X@X.T - 128*S = 64*ghat - 128*S, with the
-128*S term contributed by a small one-hot block (lhsT 16*Y, rhs -8*Y,
Y = onehot(labels)).  Then per row:
  positive dists = sqrt(relu(-P/32 - 2))     (diff-class and diagonal -> 0)
  hardest negative^2 = min over row of e, e = 2 - P/32 = dist^2 + 4*S

Host-side trick: rows are SORTED BY LABEL, so each row's same-class
columns live in a narrow diagonal band.  The sqrt/accumulate positive
pass (ACT engine) then only touches a ~(128+2*maxc)-wide column window
per 128-row m-tile instead of all 4096 columns, and the one-hot GEMM
blocks are only emitted for the 2-3 slabs that windows touch.  Sorting
is a symmetric permutation of dist: per-row stats permute with it and
the final mean is unchanged.

Sharding: rows split 512/core (data parallel).  Every core holds all
4096 columns as 8 fp8 slabs of 512 cols; slab order is rotated per core
so slab 0 contains its own shard columns (the matmul stationary
operand) and the label-sorted diagonal windows sit at a core-invariant
position - one SPMD program for all 8 cores.

GEMM runs in fp8 DoubleRow perf mode: each matmul contracts TWO
128-row k-chunks at 0.5 cycles/row - 4 matmuls per (m-tile, 512-col
slab) in bf16 become 2.  Chunks are processed column-pair-major so slab
j is first needed ~j/8 of the way through the GEMM, hiding the HBM
stream (the bf16 baseline was DMA-bound: 5.3MB/core; fp8 needs 2.1MB).

The hardest-negative reduction is split across engines (DVE may read
only ONE PSUM operand per instruction and ingests PSUM at 1 elem/cycle):
chunks c0/c1 get a direct DVE tensor_reduce(max of P); chunks c2/c3 get
an ACT pass e = 2 - P/32 (monotone decreasing) written as fp16 to SBUF,
where DVE's packed 2x/4x modes make the min-reduce cheap.  The host
combines both forms, plus the windowed positive sums, into the loss.
"""

import sys

if "/opt/trn_rl_repo" not in sys.path:
    sys.path.insert(0, "/opt/trn_rl_repo")

import ml_dtypes
import numpy as np

import concourse.bass as bass
import concourse.bacc as bacc
import concourse.mybir as mybir
import concourse.tile as tile
from concourse.bass_utils import run_bass_kernel_spmd

F32 = mybir.dt.float32
F16 = mybir.dt.float16
FP8 = mybir.dt.float8e4
NP_FP8 = ml_dtypes.float8_e4m3
ALU = mybir.AluOpType
ACTF = mybir.ActivationFunctionType
AXX = mybir.AxisListType.X
PERF = mybir.MatmulPerfMode.DoubleRow

B = 4096
D = 512
C = 64
NCORES = 8
SHARD = B // NCORES          # 512 rows per core
MT = 4                       # m-tiles per core
NJ = 8                       # column slabs of 512
KC = 4                       # data k-chunks of 128 (contracted in 2 pairs)
SCALE = 8.0                  # fp8 input scale; P = 64*ghat - 128*S
ACT_MAX_CHUNKS = (2, 3)      # chunk cols handled by the ACT e-pass path

MARGIN = 1.0


def _plan(maxc):
    """Label-sorted geometry, in LOCAL (rotated) column coords.

    Window of m-tile m = all columns that can share a class with its
    rows: [128m - (maxc-1), 128m + 127 + maxc).  Returns the ACT
    positive-pass segments per 1024-col psum chunk and the (m, slab)
    pairs needing a one-hot matmul.
    """
    wins = []
    for m in range(MT):
        lo = 128 * m - (maxc - 1)
        hi = 128 * m + 128 + (maxc - 1)
        w = min(hi - lo, B)
        ivs = []
        lo %= B
        while w > 0:
            take = min(w, B - lo)
            ivs.append((lo, lo + take))
            lo = 0
            w -= take
        wins.append(ivs)

    segs = []    # (m, c, lo_in_chunk, hi_in_chunk)
    oh = set()   # (m, local slab)
    for m, ivs in enumerate(wins):
        for (a, b) in ivs:
            for c in range(4):
                clo, chi = 1024 * c, 1024 * (c + 1)
                s_lo, s_hi = max(a, clo), min(b, chi)
                if s_lo < s_hi:
                    segs.append((m, c, s_lo - clo, s_hi - clo))
            for s in range(NJ):
                if max(a, 512 * s) < min(b, 512 * (s + 1)):
                    oh.add((m, s))
    oh_slabs = sorted({s for (_, s) in oh})
    return segs, oh, oh_slabs


def _build_nc(maxc):
    segs, oh, oh_slabs = _plan(maxc)
    noh = len(oh_slabs)
    oh_idx = {s: t for t, s in enumerate(oh_slabs)}
    nstat = 16 + len(segs)

    nc = bacc.Bacc(
        "TRN2",
        target_bir_lowering=False,
        debug=False,
        enable_asserts=False,
        num_devices=NCORES,
    )
    atp0a = nc.dram_tensor("atp0a", [128, 2, 512], FP8, kind="ExternalInput")
    atp0b = nc.dram_tensor("atp0b", [128, 2, 512], FP8, kind="ExternalInput")
    atp1 = nc.dram_tensor("atp1", [128, KC, 512], FP8, kind="ExternalInput")
    atp23 = nc.dram_tensor("atp23", [128, 2, KC, 512], FP8, kind="ExternalInput")
    atp45 = nc.dram_tensor("atp45", [128, 2, KC, 512], FP8, kind="ExternalInput")
    atp67 = nc.dram_tensor("atp67", [128, 2, KC, 512], FP8, kind="ExternalInput")
    yl = nc.dram_tensor("yl", [32, 2, SHARD], FP8, kind="ExternalInput")
    yr = nc.dram_tensor("yr", [32, noh, 2, 512], FP8, kind="ExternalInput")
    stats_d = nc.dram_tensor("stats", [128, nstat], F32, kind="ExternalOutput")

    with tile.TileContext(nc) as tc:
        with (
            tc.tile_pool(name="slabs", bufs=1) as slabs,
            tc.tile_pool(name="psum", bufs=4, space=bass.MemorySpace.PSUM) as psum,
            tc.tile_pool(name="scr", bufs=1) as scr,
            tc.tile_pool(name="esb", bufs=3) as esbp,
            tc.tile_pool(name="stat", bufs=1) as stat,
        ):
            # --- SBUF tiles -------------------------------------------------
            s0a = slabs.tile([128, 2, 512], FP8, name="s0a", tag="s0a")
            s0b = slabs.tile([128, 2, 512], FP8, name="s0b", tag="s0b")
            s1 = slabs.tile([128, KC, 512], FP8, name="s1", tag="s1")
            s23 = slabs.tile([128, 2, KC, 512], FP8, name="s23", tag="s23")
            s45 = slabs.tile([128, 2, KC, 512], FP8, name="s45", tag="s45")
            s67 = slabs.tile([128, 2, KC, 512], FP8, name="s67", tag="s67")
            ylt = stat.tile([32, 2, SHARD], FP8, name="ylt", tag="ylt")
            yrt = stat.tile([32, noh, 2, 512], FP8, name="yrt", tag="yrt")
            parts = stat.tile([128, nstat], F32, name="parts", tag="parts")
            # ACT window scratch
            wt = scr.tile([128, 1024], F32, name="wt", tag="wt")
            dsc = scr.tile([128, 1024], F32, name="dsc", tag="dsc")

            # --- DMA issue: sync queue carries the early slabs --------------
            nc.sync.dma_start(s0a[:], atp0a.ap())
            nc.sync.dma_start(s0b[:], atp0b.ap())
            nc.sync.dma_start(s1[:], atp1.ap())
            # scalar (Activation HWDGE) queue: one-hot blocks + mid slabs
            nc.scalar.dma_start(ylt[:], yl.ap())
            nc.scalar.dma_start(yrt[:], yr.ap())
            nc.scalar.dma_start(s23[:], atp23.ap())
            # gpsimd (SWDGE) queue: late slabs
            nc.gpsimd.dma_start(s45[:], atp45.ap())
            nc.gpsimd.dma_start(s67[:], atp67.ap())

            # --- constants & warm-up ---------------------------------------
            bias_c = {}
            for bname, bval in [("m2", -2.0), ("p2", 2.0), ("z", 0.0)]:
                bt = stat.tile([128, 1], F32, name=f"bc_{bname}", tag=f"bc_{bname}")
                nc.gpsimd.memset(bt[:], bval)
                bias_c[bname] = bt

            warm = stat.tile([128, 1], F32, name="warm", tag="warm")
            nc.scalar.activation(warm[:], bias_c["z"][:], ACTF.Relu)
            nc.scalar.activation(warm[:], warm[:], ACTF.Sqrt, bias=bias_c["z"][:])

            # PE warm-up: dummy matmuls open the HAM clock gate / p-state
            # ramp while the first slab DMA is in flight
            warm_w = stat.tile([128, 2, 128], FP8, name="warm_w", tag="warm_w")
            warm_x = stat.tile([128, 2, 512], FP8, name="warm_x", tag="warm_x")
            nc.gpsimd.memset(warm_w[:], 0.0)
            nc.gpsimd.memset(warm_x[:], 0.0)
            wpt = psum.tile([128, 512], F32, name="wpt", tag="pt")
            for _ in range(10):
                nc.tensor.matmul(
                    wpt[:], warm_w[:], warm_x[:], start=True, stop=True,
                    perf_mode=PERF,
                )

            # --- main loop: column-pair-major over (chunk, m-tile) ----------
            def rhs_ap(s, kk):
                if s == 0:
                    return (s0a if kk == 0 else s0b)[:, :, :]
                if s == 1:
                    return s1[:, 2 * kk : 2 * kk + 2, :]
                t = (s23, s45, s67)[s // 2 - 1]
                return t[:, s % 2, 2 * kk : 2 * kk + 2, :]

            segcol = {}
            for i, (m, c, lo, hi) in enumerate(segs):
                segcol[(m, c, lo, hi)] = 16 + i

            for c in range(4):
                for m in range(MT):
                    pt = psum.tile([128, 1024], F32, name="pt", tag="pt")
                    for kk in range(2):
                        for sj in range(2):
                            s = 2 * c + sj
                            last = kk == 1 and (m, s) not in oh
                            nc.tensor.matmul(
                                pt[:, sj * 512 : (sj + 1) * 512],
                                (s0a if kk == 0 else s0b)[
                                    :, :, m * 128 : (m + 1) * 128
                                ],
                                rhs_ap(s, kk),
                                start=(kk == 0),
                                stop=last,
                                perf_mode=PERF,
                            )
                    for sj in range(2):
                        s = 2 * c + sj
                        if (m, s) in oh:
                            nc.tensor.matmul(
                                pt[:, sj * 512 : (sj + 1) * 512],
                                ylt[:, :, m * 128 : (m + 1) * 128],
                                yrt[:, oh_idx[s], :, :],
                                start=False,
                                stop=True,
                                perf_mode=PERF,
                            )
                    # positive pass: dist = sqrt(relu(-P/32 - 2)) over the
                    # diagonal window; accum_out emits the row-sum for free
                    for (m_, c_, lo, hi) in segs:
                        if m_ != m or c_ != c:
                            continue
                        w = hi - lo
                        col = segcol[(m_, c_, lo, hi)]
                        nc.scalar.activation(
                            wt[:, :w], pt[:, lo:hi], ACTF.Relu,
                            bias=bias_c["m2"][:], scale=-1.0 / 32.0,
                        )
                        nc.scalar.activation(
                            dsc[:, :w], wt[:, :w], ACTF.Sqrt,
                            bias=bias_c["z"][:],
                            accum_out=parts[:, col : col + 1],
                        )
                    # hardest-negative reduction, split by chunk column
                    mcol = 4 * m + c
                    if c in ACT_MAX_CHUNKS:
                        # ACT: e = 2 - P/32 (fp16, SBUF); DVE: rowmin(e)
                        et = esbp.tile([128, 1024], F16, name="et", tag="et")
                        nc.scalar.activation(
                            et[:], pt[:], ACTF.Relu,
                            bias=bias_c["p2"][:], scale=-1.0 / 32.0,
                        )
                        nc.vector.tensor_reduce(
                            parts[:, mcol : mcol + 1], et[:],
                            axis=AXX, op=ALU.min,
                        )
                    else:
                        # DVE direct: rowmax(P) from PSUM
                        nc.vector.tensor_reduce(
                            parts[:, mcol : mcol + 1], pt[:],
                            axis=AXX, op=ALU.max,
                        )

            nc.sync.dma_start(stats_d.ap(), parts[:])

    nc.compile()
    return nc, segs, oh_slabs, nstat


_NC_CACHE: dict = {}


def _get_nc(maxc):
    if maxc not in _NC_CACHE:
        _NC_CACHE[maxc] = _build_nc(maxc)
    return _NC_CACHE[maxc]


def _prep_inputs(embeddings: np.ndarray, labels: np.ndarray):
    E = np.asarray(embeddings, dtype=np.float32)
    L = np.asarray(labels).astype(np.int64)
    assert E.shape == (B, D) and L.shape == (B,)

    order = np.argsort(L, kind="stable")
    Ls = L[order]
    nrm = np.maximum(np.linalg.norm(E, axis=1), 1e-12)
    N = (E / nrm[:, None]).astype(np.float32)[order]

    cnt = np.bincount(Ls, minlength=C)
    maxc = int(cnt.max())
    nc, segs, oh_slabs, nstat = _get_nc(maxc)

    X8 = np.ascontiguousarray((SCALE * N).T.astype(NP_FP8))       # [D, B]
    # S[g][p][c][x] = X8[128c + p, 512g + x]
    S = np.ascontiguousarray(
        X8.reshape(KC, 128, NJ, 512).transpose(2, 1, 0, 3)
    )                                                             # [g,p,c,x]
    Y = (Ls[None, :] == np.arange(C, dtype=np.int64)[:, None]).astype(np.float32)

    in_maps = []
    for r in range(NCORES):
        Sr = np.roll(S, -r, axis=0)                               # local j
        rows = slice(SHARD * r, SHARD * (r + 1))
        ylr = np.ascontiguousarray(
            (2 * SCALE * Y[:, rows]).reshape(2, 32, SHARD)
            .transpose(1, 0, 2).astype(NP_FP8)
        )
        yrr = np.stack(
            [
                (-SCALE * Y[:, 512 * ((r + s) % NJ) : 512 * ((r + s) % NJ) + 512])
                .reshape(2, 32, 512)
                for s in oh_slabs
            ]
        )                                                         # [t,h,p,x]
        yrr = np.ascontiguousarray(yrr.transpose(2, 0, 1, 3).astype(NP_FP8))
        in_maps.append(
            {
                "atp0a": np.ascontiguousarray(Sr[0][:, 0:2, :]),
                "atp0b": np.ascontiguousarray(Sr[0][:, 2:4, :]),
                "atp1": np.ascontiguousarray(Sr[1]),
                "atp23": np.ascontiguousarray(Sr[2:4].transpose(1, 0, 2, 3)),
                "atp45": np.ascontiguousarray(Sr[4:6].transpose(1, 0, 2, 3)),
                "atp67": np.ascontiguousarray(Sr[6:8].transpose(1, 0, 2, 3)),
                "yl": ylr,
                "yr": yrr,
            }
        )

    pos_cnt = cnt[Ls] - 1
    neg_cnt = B - cnt[Ls]
    invc = (1.0 / np.maximum(pos_cnt, 1)).astype(np.float32)
    valid = ((pos_cnt > 0) & (neg_cnt > 0)).astype(np.float32)
    return nc, segs, nstat, in_maps, (invc, valid)


def _finish(results, segs, nstat, aux):
    invc, valid = aux
    pos_sum = np.empty(B, dtype=np.float32)
    neg2 = np.empty(B, dtype=np.float32)
    dve_cols = [c for c in range(4) if c not in ACT_MAX_CHUNKS]
    for r in range(NCORES):
        st = np.asarray(results[r]["stats"])                      # [128, nstat]
        grid = st[:, :16].reshape(128, MT, 4)
        mx = grid[:, :, dve_cols].max(axis=2)                     # rowmax(P)
        emin = grid[:, :, list(ACT_MAX_CHUNKS)].min(axis=2)       # rowmin(e)
        n2 = np.minimum(2.0 - mx / 32.0, emin)                    # [128, m]
        ps = np.zeros((128, MT), dtype=np.float32)
        for i, (m, c, lo, hi) in enumerate(segs):
            ps[:, m] += st[:, 16 + i]
        rows = slice(SHARD * r, SHARD * (r + 1))
        pos_sum[rows] = ps.T.reshape(SHARD)
        neg2[rows] = n2.T.reshape(SHARD)
    pos_stat = pos_sum * invc
    neg_stat = np.sqrt(np.maximum(neg2, 0.0), dtype=np.float32)
    per_row = np.maximum(pos_stat - neg_stat + MARGIN, 0.0) * valid
    n_valid = float(valid.sum())
    total = float(per_row.sum(dtype=np.float32))
    out = total / max(n_valid, 1.0) if n_valid > 0 else 0.0
    return np.array(out, dtype=np.float32)


def kernel(embeddings, labels, _run_kwargs=None):
    nc, segs, nstat, in_maps, aux = _prep_inputs(embeddings, labels)
    res = run_bass_kernel_spmd(
        nc, in_maps, core_ids=list(range(NCORES)), **(_run_kwargs or {})
    )
    out = _finish(res.results, segs, nstat, aux)
    if _run_kwargs:
        return out, res
    return out


# revision 16
# speedup vs baseline: 1.6498x; 1.1413x over previous
"""HardTripletLoss on 8 Trainium2 NeuronCores (Bass/Tile), fp8 edition.

Math
----
reference: emb = l2_normalize(embeddings); dist = cdist(emb, emb);
  pos_stat[i] = mean_{j: same class, j!=i} dist[i,j]
  neg_stat[i] = min_{j: diff class} dist[i,j]
  loss = mean over valid rows of relu(pos_stat - neg_stat + 1)

For unit vectors dist^2 = 2 - 2*ghat with ghat = N @ N.T.  We quantize
X = 8*N to fp8e4m3 (measured end-to-end rel err ~5e-4) and fold the
class mask into the GEMM: P = X@X.T - 128*S = 64*ghat - 128*S, with the
-128*S term contributed by a small one-hot block (lhsT 16*Y, rhs -8*Y,
Y = onehot(labels)).  Then per row:
  positive dists = sqrt(relu(-P/32 - 2))     (diff-class and diagonal -> 0)
  hardest negative^2 = min over row of e, e = 2 - P/32 = dist^2 + 4*S

Host-side trick: rows are SORTED BY LABEL, so each row's same-class
columns live in a narrow diagonal band.  The sqrt/accumulate positive
pass (ACT engine) then only touches a ~(128+2*maxc)-wide column window
per 128-row m-tile instead of all 4096 columns, and the one-hot GEMM
blocks are only emitted for the 2-3 slabs that windows touch.  Sorting
is a symmetric permutation of dist: per-row stats permute with it and
the final mean is unchanged.

Sharding: rows split 512/core (data parallel).  Every core holds all
4096 columns as 8 fp8 slabs of 512 cols; slab order is rotated per core
so slab 0 contains its own shard columns (the matmul stationary
operand) and the label-sorted diagonal windows sit at a core-invariant
position - one SPMD program for all 8 cores.

GEMM runs in fp8 DoubleRow perf mode: each matmul contracts TWO
128-row k-chunks at 0.5 cycles/row - measured 215ns per [128,512]
DoubleRow matmul vs 259ns per half-the-work bf16 matmul.  Chunks are
processed column-pair-major so slab j is first needed ~j/8 of the way
through the GEMM, hiding the HBM stream behind compute (the bf16
baseline was DMA-bound: 5.3MB/core at ~95GB/s; fp8 needs 2.2MB).

The hardest-negative reduction is split across three engines (DVE may
read only ONE PSUM operand per instruction and ingests PSUM at 1
elem/cycle; no packed fast modes materialize on HW): 11 of 16 PSUM
chunks get a direct DVE tensor_reduce(max of P); the other 5 get an ACT
pass e = 2 - P/32 (monotone decreasing) written as fp16 to SBUF, which
the otherwise-idle GPSIMD engine min-reduces.  Host combines both
forms, plus the windowed positive sums, into the loss.
"""

import sys

if "/opt/trn_rl_repo" not in sys.path:
    sys.path.insert(0, "/opt/trn_rl_repo")

import ml_dtypes
import numpy as np

import concourse.bass as bass
import concourse.bacc as bacc
import concourse.mybir as mybir
import concourse.tile as tile
from concourse.bass_utils import run_bass_kernel_spmd

F32 = mybir.dt.float32
F16 = mybir.dt.float16
FP8 = mybir.dt.float8e4
NP_FP8 = ml_dtypes.float8_e4m3
ALU = mybir.AluOpType
ACTF = mybir.ActivationFunctionType
AXX = mybir.AxisListType.X
PERF = mybir.MatmulPerfMode.DoubleRow

B = 4096
D = 512
C = 64
NCORES = 8
SHARD = B // NCORES          # 512 rows per core
MT = 4                       # m-tiles per core
NJ = 8                       # column slabs of 512
KC = 4                       # data k-chunks of 128 (contracted in 2 pairs)
SCALE = 8.0                  # fp8 input scale; P = 64*ghat - 128*S
# (c, m) psum chunks routed through the ACT e-pass (e = 2 - P/32, fp16
# SBUF) instead of a direct DVE max; per-m TT-min chains on DVE then
# fold the fp16 e-chunks at 2 elem/cycle
ACT_MAX_CHUNKS = {
    (0, 0), (1, 0), (2, 0), (3, 0),
    (0, 1), (1, 1), (2, 1), (3, 1),
    (0, 2), (1, 2),
}

MARGIN = 1.0


def _plan(maxc):
    """Label-sorted geometry, in LOCAL (rotated) column coords.

    Window of m-tile m = all columns that can share a class with its
    rows: [128m - (maxc-1), 128m + 127 + maxc).  Returns the ACT
    positive-pass segments per 1024-col psum chunk and the (m, slab)
    pairs needing a one-hot matmul.
    """
    wins = []
    for m in range(MT):
        lo = 128 * m - (maxc - 1)
        hi = 128 * m + 128 + (maxc - 1)
        w = min(hi - lo, B)
        ivs = []
        lo %= B
        while w > 0:
            take = min(w, B - lo)
            ivs.append((lo, lo + take))
            lo = 0
            w -= take
        wins.append(ivs)

    segs = []    # (m, c, lo_in_chunk, hi_in_chunk)
    oh = set()   # (m, local slab)
    for m, ivs in enumerate(wins):
        for (a, b) in ivs:
            for c in range(4):
                clo, chi = 1024 * c, 1024 * (c + 1)
                s_lo, s_hi = max(a, clo), min(b, chi)
                if s_lo < s_hi:
                    segs.append((m, c, s_lo - clo, s_hi - clo))
            for s in range(NJ):
                if max(a, 512 * s) < min(b, 512 * (s + 1)):
                    oh.add((m, s))
    oh_slabs = sorted({s for (_, s) in oh})
    return segs, oh, oh_slabs


def _build_nc(maxc):
    segs, oh, oh_slabs = _plan(maxc)
    noh = len(oh_slabs)
    oh_idx = {s: t for t, s in enumerate(oh_slabs)}
    nstat = 16 + len(segs)

    nc = bacc.Bacc(
        "TRN2",
        target_bir_lowering=False,
        debug=False,
        enable_asserts=False,
        num_devices=NCORES,
    )
    atp0a = nc.dram_tensor("atp0a", [128, 2, 512], FP8, kind="ExternalInput")
    atp0b = nc.dram_tensor("atp0b", [128, 2, 512], FP8, kind="ExternalInput")
    atp1a = nc.dram_tensor("atp1a", [128, 2, 512], FP8, kind="ExternalInput")
    atp1b = nc.dram_tensor("atp1b", [128, 2, 512], FP8, kind="ExternalInput")
    atp23 = nc.dram_tensor("atp23", [128, 2, KC, 512], FP8, kind="ExternalInput")
    atp45 = nc.dram_tensor("atp45", [128, 2, KC, 512], FP8, kind="ExternalInput")
    atp67 = nc.dram_tensor("atp67", [128, 2, KC, 512], FP8, kind="ExternalInput")
    yl = nc.dram_tensor("yl", [32, 2, SHARD], FP8, kind="ExternalInput")
    yr = nc.dram_tensor("yr", [32, noh, 2, 512], FP8, kind="ExternalInput")
    stats_d = nc.dram_tensor("stats", [128, nstat], F32, kind="ExternalOutput")

    with tile.TileContext(nc) as tc:
        with (
            tc.tile_pool(name="slabs", bufs=1) as slabs,
            tc.tile_pool(name="psum", bufs=4, space=bass.MemorySpace.PSUM) as psum,
            tc.tile_pool(name="scr", bufs=1) as scr,
            tc.tile_pool(name="esb", bufs=10) as esbp,
            tc.tile_pool(name="chn", bufs=8) as chnp,
            tc.tile_pool(name="stat", bufs=1) as stat,
        ):
            # --- SBUF tiles -------------------------------------------------
            s0a = slabs.tile([128, 2, 512], FP8, name="s0a", tag="s0a")
            s0b = slabs.tile([128, 2, 512], FP8, name="s0b", tag="s0b")
            s1a = slabs.tile([128, 2, 512], FP8, name="s1a", tag="s1a")
            s1b = slabs.tile([128, 2, 512], FP8, name="s1b", tag="s1b")
            s23 = slabs.tile([128, 2, KC, 512], FP8, name="s23", tag="s23")
            s45 = slabs.tile([128, 2, KC, 512], FP8, name="s45", tag="s45")
            s67 = slabs.tile([128, 2, KC, 512], FP8, name="s67", tag="s67")
            ylt = stat.tile([32, 2, SHARD], FP8, name="ylt", tag="ylt")
            yrt = stat.tile([32, noh, 2, 512], FP8, name="yrt", tag="yrt")
            parts = stat.tile([128, nstat], F32, name="parts", tag="parts")
            # ACT window scratch
            wt = scr.tile([128, 1024], F32, name="wt", tag="wt")
            dsc = scr.tile([128, 1024], F32, name="dsc", tag="dsc")

            # --- DMA issue on the two HWDGE queues, in order of need --------
            nc.sync.dma_start(s0a[:], atp0a.ap())
            nc.sync.dma_start(s1a[:], atp1a.ap())
            nc.sync.dma_start(s0b[:], atp0b.ap())
            nc.sync.dma_start(s1b[:], atp1b.ap())
            nc.scalar.dma_start(ylt[:], yl.ap())
            nc.scalar.dma_start(yrt[:], yr.ap())
            nc.scalar.dma_start(s23[:], atp23.ap())
            nc.scalar.dma_start(s45[:], atp45.ap())
            nc.scalar.dma_start(s67[:], atp67.ap())

            # --- constants & warm-up ---------------------------------------
            bias_c = {}
            for bname, bval in [("m2", -2.0), ("p2", 2.0), ("m4", -4.0), ("z", 0.0)]:
                bt = stat.tile([128, 1], F32, name=f"bc_{bname}", tag=f"bc_{bname}")
                nc.gpsimd.memset(bt[:], bval)
                bias_c[bname] = bt

            # Sqrt first: pulls in the one table set that serves both
            # Sqrt and Relu, so only a single ACT_TABLE_LOAD is paid
            warm = stat.tile([128, 1], F32, name="warm", tag="warm")
            nc.scalar.activation(warm[:], bias_c["z"][:], ACTF.Sqrt,
                                 bias=bias_c["z"][:])
            nc.scalar.activation(warm[:], warm[:], ACTF.Relu,
                                 bias=bias_c["z"][:])

            # PE warm-up: dummy matmuls open the HAM clock gate / p-state
            # ramp while the first slab DMA is in flight
            warm_w = stat.tile([128, 2, 128], FP8, name="warm_w", tag="warm_w")
            warm_x = stat.tile([128, 2, 512], FP8, name="warm_x", tag="warm_x")
            nc.gpsimd.memset(warm_w[:], 0.0)
            nc.gpsimd.memset(warm_x[:], 0.0)
            wpt = psum.tile([128, 512], F32, name="wpt", tag="pt")
            for _ in range(8):
                nc.tensor.matmul(
                    wpt[:], warm_w[:], warm_x[:], start=True, stop=True,
                    perf_mode=PERF,
                )

            # --- main loop: column-pair-major over (chunk, m-tile) ----------
            def rhs_ap(s, kk):
                if s == 0:
                    return (s0a if kk == 0 else s0b)[:, :, :]
                if s == 1:
                    return (s1a if kk == 0 else s1b)[:, :, :]
                t = (s23, s45, s67)[s // 2 - 1]
                return t[:, s % 2, 2 * kk : 2 * kk + 2, :]

            segcol = {}
            for i, (m, c, lo, hi) in enumerate(segs):
                segcol[(m, c, lo, hi)] = 16 + i

            chain = [None] * MT
            last_act_c = {}
            for (c, m) in ACT_MAX_CHUNKS:
                last_act_c[m] = max(last_act_c.get(m, -1), c)

            for c in range(4):
                for m in range(MT):
                    pt = psum.tile([128, 1024], F32, name="pt", tag="pt")
                    for kk in range(2):
                        for sj in range(2):
                            s = 2 * c + sj
                            last = kk == 1 and (m, s) not in oh
                            nc.tensor.matmul(
                                pt[:, sj * 512 : (sj + 1) * 512],
                                (s0a if kk == 0 else s0b)[
                                    :, :, m * 128 : (m + 1) * 128
                                ],
                                rhs_ap(s, kk),
                                start=(kk == 0),
                                stop=last,
                                perf_mode=PERF,
                            )
                    for sj in range(2):
                        s = 2 * c + sj
                        if (m, s) in oh:
                            nc.tensor.matmul(
                                pt[:, sj * 512 : (sj + 1) * 512],
                                ylt[:, :, m * 128 : (m + 1) * 128],
                                yrt[:, oh_idx[s], :, :],
                                start=False,
                                stop=True,
                                perf_mode=PERF,
                            )
                    # hardest-negative reduction, split by chunk route
                    et = None
                    if (c, m) in ACT_MAX_CHUNKS:
                        # ACT: e = 2 - P/32 = dist^2 + 4S to fp16 SBUF;
                        # DVE folds it into the m-tile's TT-min chain at
                        # 2 elem/cycle, one 1x final reduce per m-tile
                        et = esbp.tile([128, 1024], F16, name="et", tag="et")
                        nc.scalar.activation(
                            et[:], pt[:], ACTF.Relu,
                            bias=bias_c["p2"][:], scale=-1.0 / 32.0,
                        )
                        if chain[m] is None:
                            chain[m] = et
                        else:
                            r = chnp.tile([128, 1024], F16, name="rc", tag="rc")
                            nc.vector.tensor_tensor(
                                r[:], chain[m][:], et[:], ALU.min
                            )
                            chain[m] = r
                        if c == last_act_c[m]:
                            nc.vector.tensor_reduce(
                                parts[:, 4 * m : 4 * m + 1], chain[m][:],
                                axis=AXX, op=ALU.min,
                            )
                    else:
                        # DVE direct: rowmax(P) from PSUM
                        mcol = 4 * m + c
                        nc.vector.tensor_reduce(
                            parts[:, mcol : mcol + 1], pt[:],
                            axis=AXX, op=ALU.max,
                        )
                    # positive pass: dist = sqrt(relu(-P/32 - 2)) over the
                    # diagonal window (= sqrt(relu(e - 4)) on the e-path);
                    # accum_out emits the row-sum for free
                    for (m_, c_, lo, hi) in segs:
                        if m_ != m or c_ != c:
                            continue
                        w = hi - lo
                        col = segcol[(m_, c_, lo, hi)]
                        if et is not None:
                            nc.scalar.activation(
                                wt[:, :w], et[:, lo:hi], ACTF.Relu,
                                bias=bias_c["m4"][:],
                            )
                        else:
                            nc.scalar.activation(
                                wt[:, :w], pt[:, lo:hi], ACTF.Relu,
                                bias=bias_c["m2"][:], scale=-1.0 / 32.0,
                            )
                        nc.scalar.activation(
                            dsc[:, :w], wt[:, :w], ACTF.Sqrt,
                            bias=bias_c["z"][:],
                            accum_out=parts[:, col : col + 1],
                        )

            nc.sync.dma_start(stats_d.ap(), parts[:])

    nc.compile()
    return nc, segs, oh_slabs, nstat


_NC_CACHE: dict = {}


def _get_nc(maxc):
    if maxc not in _NC_CACHE:
        _NC_CACHE[maxc] = _build_nc(maxc)
    return _NC_CACHE[maxc]


def _prep_inputs(embeddings: np.ndarray, labels: np.ndarray):
    E = np.asarray(embeddings, dtype=np.float32)
    L = np.asarray(labels).astype(np.int64)
    assert E.shape == (B, D) and L.shape == (B,)

    order = np.argsort(L, kind="stable")
    Ls = L[order]
    nrm = np.maximum(np.linalg.norm(E, axis=1), 1e-12)
    N = (E / nrm[:, None]).astype(np.float32)[order]

    cnt = np.bincount(Ls, minlength=C)
    maxc = int(cnt.max())
    nc, segs, oh_slabs, nstat = _get_nc(maxc)

    X8 = np.ascontiguousarray((SCALE * N).T.astype(NP_FP8))       # [D, B]
    # S[g][p][c][x] = X8[128c + p, 512g + x]
    S = np.ascontiguousarray(
        X8.reshape(KC, 128, NJ, 512).transpose(2, 1, 0, 3)
    )                                                             # [g,p,c,x]
    Y = (Ls[None, :] == np.arange(C, dtype=np.int64)[:, None]).astype(np.float32)

    in_maps = []
    for r in range(NCORES):
        Sr = np.roll(S, -r, axis=0)                               # local j
        rows = slice(SHARD * r, SHARD * (r + 1))
        ylr = np.ascontiguousarray(
            (2 * SCALE * Y[:, rows]).reshape(2, 32, SHARD)
            .transpose(1, 0, 2).astype(NP_FP8)
        )
        yrr = np.stack(
            [
                (-SCALE * Y[:, 512 * ((r + s) % NJ) : 512 * ((r + s) % NJ) + 512])
                .reshape(2, 32, 512)
                for s in oh_slabs
            ]
        )                                                         # [t,h,p,x]
        yrr = np.ascontiguousarray(yrr.transpose(2, 0, 1, 3).astype(NP_FP8))
        in_maps.append(
            {
                "atp0a": np.ascontiguousarray(Sr[0][:, 0:2, :]),
                "atp0b": np.ascontiguousarray(Sr[0][:, 2:4, :]),
                "atp1a": np.ascontiguousarray(Sr[1][:, 0:2, :]),
                "atp1b": np.ascontiguousarray(Sr[1][:, 2:4, :]),
                "atp23": np.ascontiguousarray(Sr[2:4].transpose(1, 0, 2, 3)),
                "atp45": np.ascontiguousarray(Sr[4:6].transpose(1, 0, 2, 3)),
                "atp67": np.ascontiguousarray(Sr[6:8].transpose(1, 0, 2, 3)),
                "yl": ylr,
                "yr": yrr,
            }
        )

    pos_cnt = cnt[Ls] - 1
    neg_cnt = B - cnt[Ls]
    invc = (1.0 / np.maximum(pos_cnt, 1)).astype(np.float32)
    valid = ((pos_cnt > 0) & (neg_cnt > 0)).astype(np.float32)
    return nc, segs, nstat, in_maps, (invc, valid)


def _finish(results, segs, nstat, aux):
    invc, valid = aux
    pos_sum = np.empty(B, dtype=np.float32)
    neg2 = np.empty(B, dtype=np.float32)
    for r in range(NCORES):
        st = np.asarray(results[r]["stats"])                      # [128, nstat]
        grid = st[:, :16].reshape(128, MT, 4)
        act_ms = {m for (_, m) in ACT_MAX_CHUNKS}
        n2 = np.full((128, MT), np.inf, dtype=np.float32)
        for m in range(MT):
            if m in act_ms:
                # the m-tile's TT-min chain result lands in col 4m+0
                n2[:, m] = np.minimum(n2[:, m], grid[:, m, 0])
            for c in range(4):
                if (c, m) in ACT_MAX_CHUNKS or (c == 0 and m in act_ms):
                    continue
                n2[:, m] = np.minimum(n2[:, m], 2.0 - grid[:, m, c] / 32.0)
        ps = np.zeros((128, MT), dtype=np.float32)
        for i, (m, c, lo, hi) in enumerate(segs):
            ps[:, m] += st[:, 16 + i]
        rows = slice(SHARD * r, SHARD * (r + 1))
        pos_sum[rows] = ps.T.reshape(SHARD)
        neg2[rows] = n2.T.reshape(SHARD)
    pos_stat = pos_sum * invc
    neg_stat = np.sqrt(np.maximum(neg2, 0.0), dtype=np.float32)
    per_row = np.maximum(pos_stat - neg_stat + MARGIN, 0.0) * valid
    n_valid = float(valid.sum())
    total = float(per_row.sum(dtype=np.float32))
    out = total / max(n_valid, 1.0) if n_valid > 0 else 0.0
    return np.array(out, dtype=np.float32)


def kernel(embeddings, labels, _run_kwargs=None):
    nc, segs, nstat, in_maps, aux = _prep_inputs(embeddings, labels)
    res = run_bass_kernel_spmd(
        nc, in_maps, core_ids=list(range(NCORES)), **(_run_kwargs or {})
    )
    out = _finish(res.results, segs, nstat, aux)
    if _run_kwargs:
        return out, res
    return out
